# revision 16
# baseline (speedup 1.0000x reference)
"""CXTRNN recurrence kernel for 8 Trainium2 NeuronCores.

Math (per reference):
    inp = einsum('tbs,hs->tbh', s, W_in) + b_in
    g   = sigmoid(einsum('tbz,rz->tbr', z, W_nm) + b_nm)
    x_t = (1-a)*x_{t-1} + a*(U @ (g_t * (V^T tanh(x_{t-1}))) + inp_t)
    y   = einsum('tbh,yh->tby', xs, W_out) + b_out

Sharding: data-parallel over batch B=256 -> 32 per core; params replicated;
the T=2048 sequential loop runs locally per core.

On-device layout is [feature, batch] (features on SBUF partitions).  The
host pre-transposes s and z (and appends a ones-row so the biases b_in/b_nm
fold into the matmuls) so the device never transposes anything.
"""

import numpy as np

import concourse.bass as bass
import concourse.mybir as mybir
from concourse import bacc
from concourse.bass import ts
from concourse.bass_utils import run_bass_kernel_spmd
from concourse.tile import TileContext

T = 2048
B = 256
DIM_S, DIM_Y, DIM_Z, RANK, DIM_HID = 32, 32, 16, 16, 128
ALPHA = 0.2
NCORES = 8
BL = B // NCORES            # 32 batch elements per core
CH = 16                     # timesteps per chunk
NCHUNK = T // CH            # 128
NCOLS = CH * BL             # 512 columns per chunk tile
KS = DIM_S + 1              # 33: ones row (b_in) + s rows
KZ = DIM_Z + 1              # 17: ones row (b_nm) + z rows
# Fused-update rhs row layout: [r (0:16); zero pad (16:32); ones+s (32:65)].
# Engine accesses must start at a 32-aligned partition, so both DVE writers
# (the gate at row 0, the staging copy at row 32) land on legal offsets; the
# pad rows pair with zero weight rows.
KSR = 32 + KS               # 65

F32 = mybir.dt.float32

_BUILT = {}


def _build_module():
    """Trace the per-core Bass/Tile module (same NEFF on all 8 cores).

    Bacc (not raw Bass) is required: its compile() pass splits multi-wait
    instructions into event semaphores — hardware allows at most one
    semaphore wait per engine instruction.
    """
    nc = bacc.Bacc(None)

    s_aug = nc.dram_tensor("s_aug", [KS, T * BL], F32, kind="ExternalInput")
    z_aug = nc.dram_tensor("z_aug", [KZ, T * BL], F32, kind="ExternalInput")
    v_w = nc.dram_tensor("v_w", [DIM_HID, RANK], F32, kind="ExternalInput")
    wsr = nc.dram_tensor("wsr", [KSR, DIM_HID], F32, kind="ExternalInput")
    wnm = nc.dram_tensor("wnm", [KZ, RANK], F32, kind="ExternalInput")
    wout = nc.dram_tensor("wout", [DIM_HID, DIM_Y], F32, kind="ExternalInput")
    bout = nc.dram_tensor("bout", [DIM_Y, 1], F32, kind="ExternalInput")
    y_out = nc.dram_tensor("y_out", [DIM_Y, T * BL], F32, kind="ExternalOutput")

    AF = mybir.ActivationFunctionType
    OP = mybir.AluOpType

    with TileContext(nc) as tc:
        with (
            tc.tile_pool(name="consts", bufs=1) as consts,
            tc.tile_pool(name="sr_in", bufs=3) as sr_pool,
            tc.tile_pool(name="s_stage", bufs=3) as s_stage_pool,
            tc.tile_pool(name="z_in", bufs=3) as z_pool,
            tc.tile_pool(name="g_buf", bufs=3) as g_pool,
            tc.tile_pool(name="xs_buf", bufs=3) as xs_pool,
            tc.tile_pool(name="h_buf", bufs=3) as h_pool,
            tc.tile_pool(name="y_buf", bufs=3) as y_pool,
            tc.tile_pool(name="ps_q", bufs=2, space="PSUM") as q_psum,
            tc.tile_pool(name="ps_m", bufs=2, space="PSUM") as m_psum,
            tc.tile_pool(name="ps_g", bufs=2, space="PSUM") as g_psum,
            tc.tile_pool(name="ps_y", bufs=2, space="PSUM") as y_psum,
        ):
            v_t = consts.tile([DIM_HID, RANK], F32)
            nc.gpsimd.dma_start(v_t[:], v_w[:])
            wsr_t = consts.tile([KSR, DIM_HID], F32)
            nc.gpsimd.dma_start(wsr_t[:], wsr[:])
            wnm_t = consts.tile([KZ, RANK], F32)
            nc.gpsimd.dma_start(wnm_t[:], wnm[:])
            wout_t = consts.tile([DIM_HID, DIM_Y], F32)
            nc.gpsimd.dma_start(wout_t[:], wout[:])
            bout_t = consts.tile([DIM_Y, 1], F32)
            nc.gpsimd.dma_start(bout_t[:], bout[:])
            x_init = consts.tile([DIM_HID, BL], F32)
            nc.vector.memset(x_init[:], 0.0)

            # Matmult instructions can carry at most ONE semaphore wait
            # (single slot in the LDWEIGHTS struct).  Warm up the PE's view
            # of each weight-DMA semaphore with a throwaway matmul whose only
            # dependency is that weight tile, so the real matmuls below only
            # ever wait on their rhs producer.
            warm_ps = m_psum.tile([DIM_HID, DIM_HID], F32, tag="m_ps")
            nc.tensor.matmul(
                warm_ps[0:RANK, 0:RANK], v_t[:], v_t[:], start=True, stop=True
            )
            nc.tensor.matmul(
                warm_ps[:, 0:DIM_HID], wsr_t[:], wsr_t[:], start=True, stop=True
            )
            nc.tensor.matmul(
                warm_ps[0:RANK, 0:RANK], wnm_t[:], wnm_t[:], start=True, stop=True
            )
            nc.tensor.matmul(
                warm_ps[0:DIM_Y, 0:DIM_Y], wout_t[:], wout_t[:],
                start=True, stop=True,
            )

            x_prev = x_init[:]
            for c in range(NCHUNK):
                # ---- bulk per-chunk work (runs ahead of the recurrence) ----
                z_t = z_pool.tile([KZ, NCOLS], F32)
                nc.gpsimd.dma_start(z_t[:], z_aug[:, ts(c, NCOLS)])
                g_ps = g_psum.tile([RANK, NCOLS], F32)
                nc.tensor.matmul(g_ps[:], wnm_t[:], z_t[:], start=True, stop=True)
                g_t = g_pool.tile([RANK, NCOLS], F32)
                nc.scalar.activation(g_t[:], g_ps[:], AF.Sigmoid)

                # s chunk (with its leading ones row) lands in rows 16:49 of
                # the fused-update rhs tile; the gated rank vector r is
                # written per-step into rows 0:16.  The chunk goes through a
                # DVE staging copy so every writer of rt is the DVE — the
                # consuming matmul then needs only one semaphore wait.
                s_stage = s_stage_pool.tile([KS, NCOLS], F32)
                nc.gpsimd.dma_start(s_stage[:], s_aug[:, ts(c, NCOLS)])
                rt = sr_pool.tile([KSR, NCOLS], F32)
                nc.vector.memset(rt[0:32, :], 0.0)
                # A non-zero partition start may span at most 32 partitions,
                # so the 33-row staging copy is split 32 + 1.
                nc.vector.tensor_copy(rt[32:64, :], s_stage[0:32, :])
                nc.vector.tensor_copy(rt[64:KSR, :], s_stage[32:KS, :])

                xs_t = xs_pool.tile([DIM_HID, NCOLS], F32)

                # ---- the sequential recurrence ----
                for j in range(CH):
                    col = ts(j, BL)
                    h_t = h_pool.tile([DIM_HID, BL], F32)
                    nc.scalar.activation(h_t[:], x_prev, AF.Tanh)
                    q_ps = q_psum.tile([RANK, BL], F32)
                    nc.tensor.matmul(q_ps[:], v_t[:], h_t[:], start=True, stop=True)
                    nc.vector.tensor_tensor(
                        rt[0:RANK, col], q_ps[:], g_t[:, col], op=OP.mult
                    )
                    m_ps = m_psum.tile([DIM_HID, BL], F32)
                    nc.tensor.matmul(
                        m_ps[:], wsr_t[:], rt[:, col], start=True, stop=True
                    )
                    # x_new = (1-a)*x_prev + m   (m already carries the a-scaling)
                    nc.vector.scalar_tensor_tensor(
                        xs_t[:, col],
                        x_prev,
                        1.0 - ALPHA,
                        m_ps[:],
                        op0=OP.mult,
                        op1=OP.add,
                    )
                    x_prev = xs_t[:, col]

                # ---- emit y for the chunk ----
                y_ps = y_psum.tile([DIM_Y, NCOLS], F32)
                nc.tensor.matmul(y_ps[:], wout_t[:], xs_t[:], start=True, stop=True)
                y_t = y_pool.tile([DIM_Y, NCOLS], F32)
                nc.scalar.activation(
                    y_t[:], y_ps[:], AF.Identity, bias=bout_t[:, 0:1]
                )
                nc.sync.dma_start(y_out[:, ts(c, NCOLS)], y_t[:])

    # Bacc lowering (register allocation, event-semaphore wait splitting).
    nc.finalize()
    return nc


def _get_module():
    if "nc" not in _BUILT:
        _BUILT["nc"] = _build_module()
    return _BUILT["nc"]


def _prep_core_inputs(s, z, U, V, W_in, b_in, W_out, b_out, W_nm, b_nm, core):
    b0 = core * BL
    b1 = b0 + BL
    # [T, BL, S] -> [S, T*BL] with column index = t*BL + b
    sT = np.ascontiguousarray(
        s[:, b0:b1, :].transpose(2, 0, 1).reshape(DIM_S, T * BL)
    )
    zT = np.ascontiguousarray(
        z[:, b0:b1, :].transpose(2, 0, 1).reshape(DIM_Z, T * BL)
    )
    ones = np.ones((1, T * BL), dtype=np.float32)
    s_aug = np.concatenate([ones, sT], axis=0)
    z_aug = np.concatenate([ones, zT], axis=0)

    # lhsT for the fused update matmul: out[H, B] over K = [r; pad; 1; s]
    wsr = np.concatenate(
        [
            ALPHA * U.T,
            np.zeros((16, DIM_HID), dtype=np.float32),
            ALPHA * b_in[None, :],
            ALPHA * W_in.T,
        ],
        axis=0,
    ).astype(np.float32)
    wnm = np.concatenate([b_nm[None, :], W_nm.T], axis=0).astype(np.float32)

    return {
        "s_aug": np.ascontiguousarray(s_aug, dtype=np.float32),
        "z_aug": np.ascontiguousarray(z_aug, dtype=np.float32),
        "v_w": np.ascontiguousarray(V, dtype=np.float32),
        "wsr": np.ascontiguousarray(wsr),
        "wnm": np.ascontiguousarray(wnm),
        "wout": np.ascontiguousarray(W_out.T, dtype=np.float32),
        "bout": np.ascontiguousarray(b_out.reshape(DIM_Y, 1), dtype=np.float32),
    }


def run_sharded(inputs, trace=False):
    """Run the SPMD kernel; returns (y_full, BassKernelResults)."""
    nc = _get_module()
    in_maps = [
        _prep_core_inputs(
            inputs["s"], inputs["z"], inputs["U"], inputs["V"],
            inputs["W_in"], inputs["b_in"], inputs["W_out"], inputs["b_out"],
            inputs["W_nm"], inputs["b_nm"], core,
        )
        for core in range(NCORES)
    ]
    res = run_bass_kernel_spmd(
        nc, in_maps, core_ids=list(range(NCORES)), trace=trace
    )
    y = np.empty((T, B, DIM_Y), dtype=np.float32)
    for core in range(NCORES):
        yT = res.results[core]["y_out"]  # [Y, T*BL]
        y[:, core * BL : (core + 1) * BL, :] = (
            yT.reshape(DIM_Y, T, BL).transpose(1, 2, 0)
        )
    return y, res


def kernel(**inputs):
    inputs = {k: np.asarray(v) for k, v in inputs.items()}
    y, _ = run_sharded(inputs, trace=False)
    return y


# revision 18
# speedup vs baseline: 1.1619x; 1.1619x over previous
"""CXTRNN recurrence kernel for 8 Trainium2 NeuronCores.

Math (per reference):
    inp = einsum('tbs,hs->tbh', s, W_in) + b_in
    g   = sigmoid(einsum('tbz,rz->tbr', z, W_nm) + b_nm)
    x_t = (1-a)*x_{t-1} + a*(U @ (g_t * (V^T tanh(x_{t-1}))) + inp_t)
    y   = einsum('tbh,yh->tby', xs, W_out) + b_out

Sharding: data-parallel over batch B=256 -> 32 per core; params replicated;
the T=2048 sequential loop runs locally per core.

Design notes (driven by the TRN2 instruction cost model):
- Layout is [feature, batch]; the host pre-transposes s and z and appends
  ones-rows so biases fold into matmuls and the device never transposes.
- The per-step dependency chain is the whole ballgame (engines idle
  otherwise).  It is exactly 4 links: tanh (ACT) -> q=V^T h (PE) ->
  r=g*q (DVE) -> x' accumulation (PE).  The (1-a)*x term is folded into
  the PE accumulation group via a scaled-identity matmul (off the chain),
  which removes the DVE x-update from the chain.
- The state lives in PSUM (one bank per step, ping-pong); an off-chain ACT
  copy evacuates each state to SBUF both for the y-output matmul and as the
  rhs of the next step's scaled-identity matmul.
- sigmoid is computed as 0.5*tanh(0.5*w)+0.5 so ACT only ever uses the
  Tanh table (Tanh and Sigmoid never share an ACT table; each switch would
  cost a ~1.3us table reload).
- Bacc (not raw Bass) is required: its compile() pass splits multi-wait
  instructions into event semaphores (hardware allows at most one
  semaphore wait per engine instruction).
"""

import numpy as np

import concourse.mybir as mybir
from concourse import bacc
from concourse.bass import ts
from concourse.bass_utils import run_bass_kernel_spmd
from concourse.tile import TileContext

T = 2048
B = 256
DIM_S, DIM_Y, DIM_Z, RANK, DIM_HID = 32, 32, 16, 16, 128
ALPHA = 0.2
BETA = 1.0 - ALPHA
NCORES = 8
BL = B // NCORES            # 32 batch elements per core
CH = 16                     # timesteps per chunk
NCHUNK = T // CH            # 128
NCOLS = CH * BL             # 512 columns per chunk tile
KS = DIM_S + 1              # 33: ones row (b_in) + s rows
KZ = DIM_Z + 1              # 17: ones row (b_nm) + z rows
# Fused-update rhs row layout: [r (0:16); zero pad (16:32); ones+s (32:65)].
# Compute-engine accesses must start at a 32-aligned partition; the pad rows
# pair with zero weight rows and are zeroed by a per-chunk memset.
KSR = 32 + KS               # 65

F32 = mybir.dt.float32

_BUILT = {}


def _build_module():
    nc = bacc.Bacc(None)

    s_aug = nc.dram_tensor("s_aug", [KS, T * BL], F32, kind="ExternalInput")
    z_aug = nc.dram_tensor("z_aug", [KZ, T * BL], F32, kind="ExternalInput")
    v_w = nc.dram_tensor("v_w", [DIM_HID, RANK], F32, kind="ExternalInput")
    wsr = nc.dram_tensor("wsr", [KSR, DIM_HID], F32, kind="ExternalInput")
    wnm = nc.dram_tensor("wnm", [KZ, RANK], F32, kind="ExternalInput")
    wout = nc.dram_tensor("wout", [DIM_HID, DIM_Y], F32, kind="ExternalInput")
    beta_eye = nc.dram_tensor("beta_eye", [DIM_HID, DIM_HID], F32,
                              kind="ExternalInput")
    bout = nc.dram_tensor("bout", [DIM_Y, 1], F32, kind="ExternalInput")
    y_out = nc.dram_tensor("y_out", [DIM_Y, T * BL], F32, kind="ExternalOutput")

    AF = mybir.ActivationFunctionType
    OP = mybir.AluOpType

    with TileContext(nc) as tc:
        with (
            tc.tile_pool(name="consts", bufs=1) as consts,
            tc.tile_pool(name="sr_in", bufs=3) as sr_pool,
            tc.tile_pool(name="z_in", bufs=3) as z_pool,
            tc.tile_pool(name="g_buf", bufs=3) as g_pool,
            tc.tile_pool(name="tg_buf", bufs=3) as tg_pool,
            tc.tile_pool(name="xs_buf", bufs=3) as xs_pool,
            tc.tile_pool(name="h_buf", bufs=3) as h_pool,
            tc.tile_pool(name="y_buf", bufs=3) as y_pool,
            tc.tile_pool(name="ps_x", bufs=2, space="PSUM") as x_psum,
            tc.tile_pool(name="ps_q", bufs=2, space="PSUM") as q_psum,
            tc.tile_pool(name="ps_g", bufs=2, space="PSUM") as g_psum,
            tc.tile_pool(name="ps_y", bufs=2, space="PSUM") as y_psum,
        ):
            v_t = consts.tile([DIM_HID, RANK], F32)
            nc.gpsimd.dma_start(v_t[:], v_w[:])
            wsr_t = consts.tile([KSR, DIM_HID], F32)
            nc.gpsimd.dma_start(wsr_t[:], wsr[:])
            wnm_t = consts.tile([KZ, RANK], F32)
            nc.gpsimd.dma_start(wnm_t[:], wnm[:])
            wout_t = consts.tile([DIM_HID, DIM_Y], F32)
            nc.gpsimd.dma_start(wout_t[:], wout[:])
            beye_t = consts.tile([DIM_HID, DIM_HID], F32)
            nc.gpsimd.dma_start(beye_t[:], beta_eye[:])
            bout_t = consts.tile([DIM_Y, 1], F32)
            nc.gpsimd.dma_start(bout_t[:], bout[:])
            x_init = consts.tile([DIM_HID, BL], F32)
            nc.vector.memset(x_init[:], 0.0)

            def emit_y(c, xs_tile):
                y_ps = y_psum.tile([DIM_Y, NCOLS], F32, name=f"y_ps_{c}", tag="y_ps")
                nc.tensor.matmul(
                    y_ps[:], wout_t[:], xs_tile[:], start=True, stop=True
                )
                y_t = y_pool.tile([DIM_Y, NCOLS], F32, name=f"y_t_{c}", tag="y_t")
                # y = y_ps + b_out (per-partition scalar), on DVE
                nc.vector.tensor_scalar(
                    y_t[:], y_ps[:], bout_t[:, 0:1], None, op0=OP.add
                )
                nc.sync.dma_start(y_out[:, ts(c, NCOLS)], y_t[:])

            x_prev_ps = None        # PSUM bank holding x_j (state)
            x_prev_sbuf = x_init    # SBUF copy of x_{j-1}... (rhs of mm_I)
            xs_tiles = {}           # chunk -> xs sbuf tile
            rt = None
            g_t = None

            for j in range(T):
                c = j // CH
                jj = j % CH
                if jj == 0:
                    # ---- per-chunk bulk work (off the critical chain) ----
                    z_t = z_pool.tile([KZ, NCOLS], F32, name=f"z_t_{c}", tag="z_t")
                    nc.gpsimd.dma_start(z_t[:], z_aug[:, ts(c, NCOLS)])
                    g_ps = g_psum.tile([RANK, NCOLS], F32, name=f"g_ps_{c}", tag="g_ps")
                    nc.tensor.matmul(
                        g_ps[:], wnm_t[:], z_t[:], start=True, stop=True
                    )
                    # sigmoid(w) = 0.5*tanh(0.5*w) + 0.5  (stay on Tanh table)
                    tg_t = tg_pool.tile([RANK, NCOLS], F32, name=f"tg_t_{c}", tag="tg_t")
                    nc.scalar.activation(tg_t[:], g_ps[:], AF.Tanh, scale=0.5)
                    g_t = g_pool.tile([RANK, NCOLS], F32, name=f"g_t_{c}", tag="g_t")
                    nc.vector.tensor_scalar(
                        g_t[:], tg_t[:], 0.5, 0.5, op0=OP.mult, op1=OP.add
                    )

                    rt = sr_pool.tile([KSR, NCOLS], F32, name=f"rt_{c}", tag="rt")
                    # zero the r + pad rows (pad rows pair with zero weights)
                    nc.vector.memset(rt[0:32, :], 0.0)
                    # ones+s rows; engine-style 32-partition limit applies to
                    # some DMA lowerings too, so split 32 + 1.
                    nc.sync.dma_start(
                        rt[32:64, :], s_aug[0:32, ts(c, NCOLS)]
                    )
                    nc.sync.dma_start(
                        rt[64:KSR, :], s_aug[32:KS, ts(c, NCOLS)]
                    )

                    xs_tiles[c] = xs_pool.tile(
                        [DIM_HID, NCOLS], F32, name=f"xs_{c}", tag="xs"
                    )

                col = ts(jj, BL)

                # ---- critical chain: tanh -> mm1 -> gate -> mm_sr ----
                h_t = h_pool.tile([DIM_HID, BL], F32, name=f"h_{j}", tag="h")
                if x_prev_ps is None:
                    nc.scalar.activation(h_t[:], x_init[:], AF.Tanh)
                else:
                    nc.scalar.activation(h_t[:], x_prev_ps[:], AF.Tanh)

                # off-chain: evacuate x_j to SBUF (y-path + next mm_I rhs)
                if j > 0:
                    pc, pj = (j - 1) // CH, (j - 1) % CH
                    x_sb = xs_tiles[pc][:, ts(pj, BL)]
                    nc.scalar.activation(x_sb, x_prev_ps[:], AF.Copy)
                    x_prev_sbuf = x_sb
                    if pj == CH - 1:
                        emit_y(pc, xs_tiles[pc])
                        if pc - 2 in xs_tiles:
                            del xs_tiles[pc - 2]

                q_ps = q_psum.tile([RANK, BL], F32, name=f"q_{j}", tag="q")
                nc.tensor.matmul(q_ps[:], v_t[:], h_t[:], start=True, stop=True)

                nc.vector.tensor_tensor(
                    rt[0:RANK, col], q_ps[:], g_t[:, col], op=OP.mult
                )

                x_ps = x_psum.tile([DIM_HID, BL], F32, name=f"x_{j}", tag="x")
                if j > 0:
                    # beta*x_{j-1} into the bank (off-chain: only needs the
                    # SBUF evacuation of x_{j-1}), then accumulate the fused
                    # alpha*(U r + b + W s) on top.
                    nc.tensor.matmul(
                        x_ps[:], beye_t[:], x_prev_sbuf, start=True, stop=False
                    )
                    nc.tensor.matmul(
                        x_ps[:], wsr_t[:], rt[:, col], start=False, stop=True
                    )
                else:
                    nc.tensor.matmul(
                        x_ps[:], wsr_t[:], rt[:, col], start=True, stop=True
                    )
                x_prev_ps = x_ps

            # final state evacuation + last chunk's y
            last = xs_tiles[NCHUNK - 1][:, ts(CH - 1, BL)]
            nc.scalar.activation(last, x_prev_ps[:], AF.Copy)
            emit_y(NCHUNK - 1, xs_tiles[NCHUNK - 1])

    nc.finalize()
    return nc


def _get_module():
    if "nc" not in _BUILT:
        _BUILT["nc"] = _build_module()
    return _BUILT["nc"]


def _prep_core_inputs(s, z, U, V, W_in, b_in, W_out, b_out, W_nm, b_nm, core):
    b0 = core * BL
    b1 = b0 + BL
    # [T, BL, S] -> [S, T*BL] with column index = t*BL + b
    sT = np.ascontiguousarray(
        s[:, b0:b1, :].transpose(2, 0, 1).reshape(DIM_S, T * BL)
    )
    zT = np.ascontiguousarray(
        z[:, b0:b1, :].transpose(2, 0, 1).reshape(DIM_Z, T * BL)
    )
    ones = np.ones((1, T * BL), dtype=np.float32)
    s_aug = np.concatenate([ones, sT], axis=0)
    z_aug = np.concatenate([ones, zT], axis=0)

    # lhsT for the fused update matmul: out[H, B] over K = [r; pad; 1; s]
    wsr = np.concatenate(
        [
            ALPHA * U.T,
            np.zeros((16, DIM_HID), dtype=np.float32),
            ALPHA * b_in[None, :],
            ALPHA * W_in.T,
        ],
        axis=0,
    ).astype(np.float32)
    wnm = np.concatenate([b_nm[None, :], W_nm.T], axis=0).astype(np.float32)

    return {
        "s_aug": np.ascontiguousarray(s_aug, dtype=np.float32),
        "z_aug": np.ascontiguousarray(z_aug, dtype=np.float32),
        "v_w": np.ascontiguousarray(V, dtype=np.float32),
        "wsr": np.ascontiguousarray(wsr),
        "wnm": np.ascontiguousarray(wnm),
        "wout": np.ascontiguousarray(W_out.T, dtype=np.float32),
        "beta_eye": np.ascontiguousarray(
            BETA * np.eye(DIM_HID, dtype=np.float32)
        ),
        "bout": np.ascontiguousarray(b_out.reshape(DIM_Y, 1), dtype=np.float32),
    }


def run_sharded(inputs, trace=False):
    """Run the SPMD kernel; returns (y_full, BassKernelResults)."""
    nc = _get_module()
    in_maps = [
        _prep_core_inputs(
            inputs["s"], inputs["z"], inputs["U"], inputs["V"],
            inputs["W_in"], inputs["b_in"], inputs["W_out"], inputs["b_out"],
            inputs["W_nm"], inputs["b_nm"], core,
        )
        for core in range(NCORES)
    ]
    res = run_bass_kernel_spmd(
        nc, in_maps, core_ids=list(range(NCORES)), trace=trace
    )
    y = np.empty((T, B, DIM_Y), dtype=np.float32)
    for core in range(NCORES):
        yT = res.results[core]["y_out"]  # [Y, T*BL]
        y[:, core * BL : (core + 1) * BL, :] = (
            yT.reshape(DIM_Y, T, BL).transpose(1, 2, 0)
        )
    return y, res


def kernel(**inputs):
    inputs = {k: np.asarray(v) for k, v in inputs.items()}
    y, _ = run_sharded(inputs, trace=False)
    return y


# revision 32
# speedup vs baseline: 68.5702x; 59.0138x over previous
"""CXTRNN recurrence kernel for 8 Trainium2 NeuronCores.

Math (per reference):
    inp = einsum('tbs,hs->tbh', s, W_in) + b_in
    g   = sigmoid(einsum('tbz,rz->tbr', z, W_nm) + b_nm)
    x_t = (1-a)*x_{t-1} + a*(U @ (g_t * (V^T tanh(x_{t-1}))) + inp_t)
    y   = einsum('tbh,yh->tby', xs, W_out) + b_out

Sharding: data-parallel over batch B=256 -> 32 per core; params replicated;
the T=2048 sequential loop runs locally per core.

Design notes (driven by the TRN2 instruction cost model):
- Layout is [feature, batch]; the host pre-transposes s and z and appends
  ones-rows so biases fold into matmuls and the device never transposes.
- The per-step dependency chain is the whole ballgame (engines idle
  otherwise).  It is exactly 4 links: tanh (ACT) -> q=V^T h (PE) ->
  r=g*q (DVE) -> x' accumulation (PE).  The (1-a)*x term is folded into
  the PE accumulation group via a scaled-identity matmul (off the chain),
  which removes the DVE x-update from the chain.
- The state lives in PSUM (one bank per step, ping-pong); an off-chain ACT
  copy evacuates each state to SBUF both for the y-output matmul and as the
  rhs of the next step's scaled-identity matmul.
- sigmoid is computed as 0.5*tanh(0.5*w)+0.5 so ACT only ever uses the
  Tanh table (Tanh and Sigmoid never share an ACT table; each switch would
  cost a ~1.3us table reload).
- Bacc (not raw Bass) is required: its compile() pass splits multi-wait
  instructions into event semaphores (hardware allows at most one
  semaphore wait per engine instruction).
"""

import numpy as np

import concourse.mybir as mybir
from concourse import bacc
from concourse.tile_autobufs import add_dep_helper
from concourse.bass import ts
from concourse.bass_utils import run_bass_kernel_spmd
from concourse.tile import TileContext

T = 2048
B = 256
DIM_S, DIM_Y, DIM_Z, RANK, DIM_HID = 32, 32, 16, 16, 128
ALPHA = 0.2
BETA = 1.0 - ALPHA
NCORES = 8
BL = B // NCORES            # 32 batch elements per core
CH = 16                     # timesteps per chunk
NCHUNK = T // CH            # 128
NCOLS = CH * BL             # 512 columns per chunk tile
KS = DIM_S + 1              # 33: ones row (b_in) + s rows
KZ = DIM_Z + 1              # 17: ones row (b_nm) + z rows
# Fused-update rhs row layout: [r (0:16); zero pad (16:32); ones+s (32:65)].
# Compute-engine accesses must start at a 32-aligned partition; the pad rows
# pair with zero weight rows and are zeroed by a per-chunk memset.
KSR = 32 + KS               # 65

F32 = mybir.dt.float32

_BUILT = {}


def _build_module():
    nc = bacc.Bacc(None)

    s_aug = nc.dram_tensor("s_aug", [KS, T * BL], F32, kind="ExternalInput")
    z_aug = nc.dram_tensor("z_aug", [KZ, T * BL], F32, kind="ExternalInput")
    v_w = nc.dram_tensor("v_w", [DIM_HID, RANK], F32, kind="ExternalInput")
    wsr = nc.dram_tensor("wsr", [KSR, DIM_HID], F32, kind="ExternalInput")
    wnm = nc.dram_tensor("wnm", [KZ, RANK], F32, kind="ExternalInput")
    wout = nc.dram_tensor("wout", [DIM_HID, DIM_Y], F32, kind="ExternalInput")
    beta_eye = nc.dram_tensor("beta_eye", [DIM_HID, DIM_HID], F32,
                              kind="ExternalInput")
    bout = nc.dram_tensor("bout", [DIM_Y, 1], F32, kind="ExternalInput")
    y_out = nc.dram_tensor("y_out", [DIM_Y, T * BL], F32, kind="ExternalOutput")

    AF = mybir.ActivationFunctionType
    OP = mybir.AluOpType

    with TileContext(nc) as tc:
        with (
            tc.tile_pool(name="consts", bufs=1) as consts,
            tc.tile_pool(name="sr_in", bufs=4) as sr_pool,
            tc.tile_pool(name="z_in", bufs=4) as z_pool,
            tc.tile_pool(name="g_buf", bufs=4) as g_pool,
            tc.tile_pool(name="tg_buf", bufs=4) as tg_pool,
            tc.tile_pool(name="xs_buf", bufs=3) as xs_pool,
            tc.tile_pool(name="h_buf", bufs=3) as h_pool,
            tc.tile_pool(name="y_buf", bufs=3) as y_pool,
            tc.tile_pool(name="ps_x", bufs=2, space="PSUM") as x_psum,
            tc.tile_pool(name="ps_q", bufs=2, space="PSUM") as q_psum,
            tc.tile_pool(name="ps_g", bufs=2, space="PSUM") as g_psum,
            tc.tile_pool(name="ps_y", bufs=2, space="PSUM") as y_psum,
        ):
            v_t = consts.tile([DIM_HID, RANK], F32)
            nc.gpsimd.dma_start(v_t[:], v_w[:])
            wsr_t = consts.tile([KSR, DIM_HID], F32)
            nc.gpsimd.dma_start(wsr_t[:], wsr[:])
            wnm_t = consts.tile([KZ, RANK], F32)
            nc.gpsimd.dma_start(wnm_t[:], wnm[:])
            wout_t = consts.tile([DIM_HID, DIM_Y], F32)
            nc.gpsimd.dma_start(wout_t[:], wout[:])
            beye_t = consts.tile([DIM_HID, DIM_HID], F32)
            nc.gpsimd.dma_start(beye_t[:], beta_eye[:])
            bout_t = consts.tile([DIM_Y, 1], F32)
            nc.gpsimd.dma_start(bout_t[:], bout[:])
            x_init = consts.tile([DIM_HID, BL], F32)
            nc.vector.memset(x_init[:], 0.0)

            # Bulk work is emitted in staggered phases (different step slots)
            # so each in-order engine reaches a bulk op only well after its
            # dependencies completed — a bulk op with unmet deps stalls the
            # engine and with it the recurrence chain.
            state = {"zt": {}, "gps": {}, "tg": {}, "g": {}, "rt": {},
                     "yps": {}, "xs": {}}

            def bulk_dma(c):
                z_t = z_pool.tile([KZ, NCOLS], F32, name=f"z_t_{c}", tag="z_t")
                nc.sync.dma_start(z_t[:], z_aug[:, ts(c, NCOLS)])
                state["zt"][c] = z_t
                rt = sr_pool.tile([KSR, NCOLS], F32, name=f"rt_{c}", tag="rt")
                # zero the r + pad rows (pad rows pair with zero weights);
                # gpsimd so the DVE (gate engine) never stalls on it
                nc.gpsimd.memset(rt[0:32, :], 0.0)
                # ones+s rows; 32-partition limit at non-zero offsets
                nc.sync.dma_start(rt[32:64, :], s_aug[0:32, ts(c, NCOLS)])
                nc.sync.dma_start(rt[64:KSR, :], s_aug[32:KS, ts(c, NCOLS)])
                state["rt"][c] = rt

            def _after(inst, anchor):
                if anchor is not None:
                    add_dep_helper(inst.ins, anchor.ins, sync=False,
                                   reason="bulk op ordered behind chain op")

            def bulk_mmg(c, piece, anchor=None):
                if piece == 0:
                    state["gps"][c] = g_psum.tile(
                        [RANK, NCOLS], F32, name=f"g_ps_{c}", tag="g_ps"
                    )
                p = piece * 128
                mm = nc.tensor.matmul(
                    state["gps"][c][:, p : p + 128], wnm_t[:],
                    state["zt"][c][:, p : p + 128], start=True, stop=True,
                )
                _after(mm, anchor)

            def bulk_tg(c, piece, anchor=None):
                # sigmoid(w) = 0.5*tanh(0.5*w) + 0.5  (stay on Tanh table);
                # 128-col pieces so no single ACT op can block a tanh long
                if piece == 0:
                    state["tg"][c] = tg_pool.tile(
                        [RANK, NCOLS], F32, name=f"tg_t_{c}", tag="tg_t"
                    )
                p = piece * 128
                a = nc.scalar.activation(
                    state["tg"][c][:, p : p + 128],
                    state["gps"][c][:, p : p + 128], AF.Tanh, scale=0.5,
                )
                _after(a, anchor)

            def bulk_gaffine(c):
                # on gpsimd: keeps the DVE free for the chain's gate op
                g_t = g_pool.tile([RANK, NCOLS], F32, name=f"g_t_{c}", tag="g_t")
                nc.gpsimd.tensor_scalar(
                    g_t[:], state["tg"][c][:], 0.5, 0.5, op0=OP.mult, op1=OP.add
                )
                state["g"][c] = g_t

            def y_mm(c, piece, anchor=None):
                if piece == 0:
                    state["yps"][c] = y_psum.tile(
                        [DIM_Y, NCOLS], F32, name=f"y_ps_{c}", tag="y_ps"
                    )
                p = piece * 128
                mm = nc.tensor.matmul(
                    state["yps"][c][:, p : p + 128], wout_t[:],
                    state["xs"][c][:, p : p + 128], start=True, stop=True,
                )
                _after(mm, anchor)

            def y_evac(c, piece, anchor=None):
                # PSUM -> SBUF (+b_out) in 128-col pieces on DVE
                if piece == 0:
                    state["yt"] = state.get("yt", {})
                    state["yt"][c] = y_pool.tile(
                        [DIM_Y, NCOLS], F32, name=f"y_t_{c}", tag="y_t"
                    )
                p = piece * 128
                tsv = nc.vector.tensor_scalar(
                    state["yt"][c][:, p : p + 128],
                    state["yps"][c][:, p : p + 128],
                    bout_t[:, 0:1], None, op0=OP.add,
                )
                _after(tsv, anchor)

            def y_out_emit(c):
                nc.sync.dma_start(y_out[:, ts(c, NCOLS)], state["yt"][c][:])
                del state["yps"][c], state["xs"][c], state["yt"][c]

            x_prev_ps = None        # PSUM bank holding x_j (state)
            x_prev_sbuf = x_init    # SBUF copy of the previous state
            LAST = NCHUNK - 1

            # pipeline fill: bulk for the first two chunks
            for c0 in (0, 1):
                bulk_dma(c0)
                for p in range(4):
                    bulk_mmg(c0, p)
                for p in range(4):
                    bulk_tg(c0, p)
                bulk_gaffine(c0)

            def emit_offchain(c, jj, pe_a, dve_a, act_a):
                """Bulk work for step slot jj of chunk c — ordered behind
                the same step's chain op on each in-order engine so it lands
                in the idle gap behind the chain, never in front of it."""
                if jj == 0:
                    if c + 2 < NCHUNK:
                        bulk_dma(c + 2)
                elif jj in (1, 2, 3, 4) and c > 0:
                    y_mm(c - 1, jj - 1, pe_a)
                if jj in (2, 3, 4, 5) and c > 0:
                    y_evac(c - 1, jj - 2, dve_a)
                elif jj == 7 and c > 0:
                    y_out_emit(c - 1)
                if jj in (4, 5, 6, 7) and c + 2 < NCHUNK:
                    bulk_mmg(c + 2, jj - 4, pe_a)
                elif jj in (8, 10, 12, 14) and c + 2 < NCHUNK:
                    bulk_tg(c + 2, (jj - 8) // 2, act_a)
                elif jj == 15 and c + 2 < NCHUNK:
                    bulk_gaffine(c + 2)

            for j in range(T):
                c = j // CH
                jj = j % CH
                if jj == 0:
                    state["xs"][c] = xs_pool.tile(
                        [DIM_HID, NCOLS], F32, name=f"xs_{c}", tag="xs"
                    )

                rt = state["rt"][c]
                g_t = state["g"][c]
                col = ts(jj, BL)

                # ---- critical chain: tanh -> mm1 -> gate -> mm_sr ----
                h_t = h_pool.tile([DIM_HID, BL], F32, name=f"h_{j}", tag="h")
                if x_prev_ps is None:
                    th = nc.scalar.activation(h_t[:], x_init[:], AF.Tanh)
                else:
                    th = nc.scalar.activation(h_t[:], x_prev_ps[:], AF.Tanh)

                # off-chain: evacuate x_j to SBUF (y-path + next mm_I rhs)
                if j > 0:
                    pc, pj = (j - 1) // CH, (j - 1) % CH
                    x_sb = state["xs"][pc][:, ts(pj, BL)]
                    cp = nc.scalar.activation(x_sb, x_prev_ps[:], AF.Copy)
                    x_prev_sbuf = x_sb

                q_ps = q_psum.tile([RANK, BL], F32, name=f"q_{j}", tag="q")
                nc.tensor.matmul(q_ps[:], v_t[:], h_t[:], start=True, stop=True)

                gate = nc.vector.tensor_tensor(
                    rt[0:RANK, col], q_ps[:], g_t[:, col], op=OP.mult
                )

                x_ps = x_psum.tile([DIM_HID, BL], F32, name=f"x_{j}", tag="x")
                if j > 0:
                    # beta*x_{j-1} into the bank (off-chain: only needs the
                    # SBUF evacuation of x_{j-1}), then accumulate the fused
                    # alpha*(U r + b + W s) on top.
                    nc.tensor.matmul(
                        x_ps[:], beye_t[:], x_prev_sbuf, start=True, stop=False
                    )
                    mm_sr = nc.tensor.matmul(
                        x_ps[:], wsr_t[:], rt[:, col], start=False, stop=True
                    )
                else:
                    mm_sr = nc.tensor.matmul(
                        x_ps[:], wsr_t[:], rt[:, col], start=True, stop=True
                    )
                x_prev_ps = x_ps

                emit_offchain(c, jj, mm_sr, gate, cp if j > 0 else th)

            # final state evacuation + last chunk's y
            last = state["xs"][LAST][:, ts(CH - 1, BL)]
            nc.scalar.activation(last, x_prev_ps[:], AF.Copy)
            for p in range(4):
                y_mm(LAST, p)
            for p in range(4):
                y_evac(LAST, p)
            y_out_emit(LAST)

    nc.finalize()
    return nc


def _get_module():
    if "nc" not in _BUILT:
        _BUILT["nc"] = _build_module()
    return _BUILT["nc"]


def _prep_core_inputs(s, z, U, V, W_in, b_in, W_out, b_out, W_nm, b_nm, core):
    b0 = core * BL
    b1 = b0 + BL
    # [T, BL, S] -> [S, T*BL] with column index = t*BL + b
    sT = np.ascontiguousarray(
        s[:, b0:b1, :].transpose(2, 0, 1).reshape(DIM_S, T * BL)
    )
    zT = np.ascontiguousarray(
        z[:, b0:b1, :].transpose(2, 0, 1).reshape(DIM_Z, T * BL)
    )
    ones = np.ones((1, T * BL), dtype=np.float32)
    s_aug = np.concatenate([ones, sT], axis=0)
    z_aug = np.concatenate([ones, zT], axis=0)

    # lhsT for the fused update matmul: out[H, B] over K = [r; pad; 1; s]
    wsr = np.concatenate(
        [
            ALPHA * U.T,
            np.zeros((16, DIM_HID), dtype=np.float32),
            ALPHA * b_in[None, :],
            ALPHA * W_in.T,
        ],
        axis=0,
    ).astype(np.float32)
    wnm = np.concatenate([b_nm[None, :], W_nm.T], axis=0).astype(np.float32)

    return {
        "s_aug": np.ascontiguousarray(s_aug, dtype=np.float32),
        "z_aug": np.ascontiguousarray(z_aug, dtype=np.float32),
        "v_w": np.ascontiguousarray(V, dtype=np.float32),
        "wsr": np.ascontiguousarray(wsr),
        "wnm": np.ascontiguousarray(wnm),
        "wout": np.ascontiguousarray(W_out.T, dtype=np.float32),
        "beta_eye": np.ascontiguousarray(
            BETA * np.eye(DIM_HID, dtype=np.float32)
        ),
        "bout": np.ascontiguousarray(b_out.reshape(DIM_Y, 1), dtype=np.float32),
    }


def run_sharded(inputs, trace=False):
    """Run the SPMD kernel; returns (y_full, BassKernelResults)."""
    nc = _get_module()
    in_maps = [
        _prep_core_inputs(
            inputs["s"], inputs["z"], inputs["U"], inputs["V"],
            inputs["W_in"], inputs["b_in"], inputs["W_out"], inputs["b_out"],
            inputs["W_nm"], inputs["b_nm"], core,
        )
        for core in range(NCORES)
    ]
    res = run_bass_kernel_spmd(
        nc, in_maps, core_ids=list(range(NCORES)), trace=trace
    )
    y = np.empty((T, B, DIM_Y), dtype=np.float32)
    for core in range(NCORES):
        yT = res.results[core]["y_out"]  # [Y, T*BL]
        y[:, core * BL : (core + 1) * BL, :] = (
            yT.reshape(DIM_Y, T, BL).transpose(1, 2, 0)
        )
    return y, res


def kernel(**inputs):
    inputs = {k: np.asarray(v) for k, v in inputs.items()}
    y, _ = run_sharded(inputs, trace=False)
    return y
